# revision 11
# baseline (speedup 1.0000x reference)
"""Box-SDF (CAPUDF box boundary distance) Trainium2 Bass kernel — fp16 v4.

For each 3-D point x (S = 0.4):
    a_i = |x_i|                         (ACT Abs)
    b_i = relu(a_i - S) = max(a_i,S)-S  (DVE fused 2-op tensor_scalar, 4x)
    mx  = max_i a_i                     (DVE max tree)
    u   = min(mx, S) - S                (<= 0; == -(inside distance))
    b0' = b_0 + u                       (disjoint support: exact)
    d   = sqrt(b0'^2 + b1^2 + b2^2)     (= sqrt(sum b_i^2 + u^2))

Placement (measured fp16 rates): ACT = abs + 3/4 of the b1 square +
sqrt; DVE = everything else (self-read tensor_tensor squares have no
fp16 penalty); PE sums the 3 planes via identity-matmul 512-col PSUM
chunks, consuming DVE-fed planes first so the ACT square can lag one
tile; sqrt+store run per PSUM half to shorten the drain.

Tile schedule [1024, 2048, 2048, 2048, 1024] points/partition-row:
small edge tiles cut pipeline fill/drain; flat DRAM layout.

Device I/O fp16 (host converts; L2 rel err ~4e-4 vs 2e-2 gate).
Sharding: data-parallel over points across 8 NeuronCores.
"""

import sys

import numpy as np

sys.path.insert(0, "/opt/trn_rl_repo")

import concourse.bacc as bacc  # noqa: E402
import concourse.mybir as mybir  # noqa: E402
from concourse import bass_utils  # noqa: E402
from concourse.tile import TileContext  # noqa: E402

N = 8388608
NCORES = 8
NPC = N // NCORES  # 1,048,576 points per core
P = 128
KPL = NPC // P  # 8192 points per partition lane
TILES = [1024, 2048, 2048, 2048, 1024]
assert sum(TILES) == KPL
NT = len(TILES)
KMAX = max(TILES)

SIZE = 0.4
F16 = mybir.dt.float16
F32 = mybir.dt.float32
AF = mybir.ActivationFunctionType
OP = mybir.AluOpType


def build_kernel():
    nc = bacc.Bacc(
        "TRN2",
        target_bir_lowering=False,
        debug=False,
        num_devices=NCORES,
    )
    # flat planar layout: per tile, 3 planes of k columns each
    x = nc.dram_tensor("x", [P, 3 * KPL], F16, kind="ExternalInput").ap()
    eye = nc.dram_tensor("eye", [P, P], F16, kind="ExternalInput").ap()
    d = nc.dram_tensor("d", [P, KPL], F16, kind="ExternalOutput").ap()

    with TileContext(nc) as tc:
        with (
            tc.tile_pool(name="const", bufs=1) as cpool,
            tc.tile_pool(name="xtp", bufs=3) as xtp,
            tc.tile_pool(name="big", bufs=2) as big,
            tc.tile_pool(name="small", bufs=3) as small,
            tc.tile_pool(name="psum", bufs=2, space="PSUM") as pspool,
        ):
            eye_t = cpool.tile([P, P], F16)
            state = {}

            def stage_a(t):
                k = TILES[t]
                xoff = 3 * sum(TILES[:t])
                xt = xtp.tile([P, 3 * KMAX], F16, tag="xt")
                aa = big.tile([P, 3 * KMAX], F16, tag="aa")
                if t == 0:
                    for c in range(3):
                        nc.sync.dma_start(
                            out=xt[:, c * k : (c + 1) * k],
                            in_=x[:, xoff + c * k : xoff + (c + 1) * k],
                        )
                        nc.scalar.activation(
                            out=aa[:, c * k : (c + 1) * k],
                            in_=xt[:, c * k : (c + 1) * k],
                            func=AF.Abs,
                        )
                else:
                    nc.sync.dma_start(
                        out=xt[:, 0 : 3 * k], in_=x[:, xoff : xoff + 3 * k]
                    )
                    nc.scalar.activation(
                        out=aa[:, 0 : 3 * k], in_=xt[:, 0 : 3 * k], func=AF.Abs
                    )

                bb = big.tile([P, 3 * KMAX], F16, tag="bb")
                nc.vector.tensor_scalar(
                    out=bb[:, 0 : 3 * k],
                    in0=aa[:, 0 : 3 * k],
                    scalar1=SIZE,
                    scalar2=-SIZE,
                    op0=OP.max,
                    op1=OP.add,
                )
                m1 = small.tile([P, KMAX], F16, tag="m1")
                nc.vector.tensor_tensor(
                    out=m1[:, 0:k], in0=aa[:, 0:k], in1=aa[:, k : 2 * k], op=OP.max
                )
                mx = small.tile([P, KMAX], F16, tag="mx")
                nc.vector.tensor_tensor(
                    out=mx[:, 0:k],
                    in0=m1[:, 0:k],
                    in1=aa[:, 2 * k : 3 * k],
                    op=OP.max,
                )
                u = small.tile([P, KMAX], F16, tag="u")
                nc.vector.tensor_scalar(
                    out=u[:, 0:k],
                    in0=mx[:, 0:k],
                    scalar1=SIZE,
                    scalar2=-SIZE,
                    op0=OP.min,
                    op1=OP.add,
                )
                b0p = small.tile([P, KMAX], F16, tag="b0p")
                nc.vector.tensor_tensor(
                    out=b0p[:, 0:k], in0=bb[:, 0:k], in1=u[:, 0:k], op=OP.add
                )

                # DVE self-squares: b0' plane, b2 plane, last 1/4 of b1
                sq = big.tile([P, 3 * KMAX], F16, tag="sq")
                nc.vector.tensor_tensor(
                    out=sq[:, 0:k], in0=b0p[:, 0:k], in1=b0p[:, 0:k], op=OP.mult
                )
                nc.vector.tensor_tensor(
                    out=sq[:, 2 * k : 3 * k],
                    in0=bb[:, 2 * k : 3 * k],
                    in1=bb[:, 2 * k : 3 * k],
                    op=OP.mult,
                )
                q = 3 * (k // 4)  # ACT-covered prefix of the b1 plane
                nc.vector.tensor_tensor(
                    out=sq[:, k + q : 2 * k],
                    in0=bb[:, k + q : 2 * k],
                    in1=bb[:, k + q : 2 * k],
                    op=OP.mult,
                )
                state[t] = (bb, sq)

            def act_square(t):
                # ACT square of the b1-plane prefix; emitted one tile late so
                # it never queues behind the next tile's Abs with unmet deps
                k = TILES[t]
                q = 3 * (k // 4)
                bb, sq = state[t]
                nc.scalar.activation(
                    out=sq[:, k : k + q], in_=bb[:, k : k + q], func=AF.Square
                )

            def stage_b(t):
                k = TILES[t]
                doff = sum(TILES[:t])
                bb, sq = state.pop(t)
                s_ps = pspool.tile([P, KMAX], F32, tag="s_ps")
                nj = k // 512
                # DVE-fed planes first (c=0: b0', c=2: b2) across all j
                for j in range(0, k, 512):
                    for c in (0, 2):
                        nc.tensor.matmul(
                            s_ps[:, j : j + 512],
                            eye_t[:],
                            sq[:, c * k + j : c * k + j + 512],
                            start=(c == 0),
                            stop=False,
                        )
                # b1 plane: DVE-fed top chunk first, then ACT-fed chunks;
                # sqrt+store per completed psum half
                dt = small.tile([P, KMAX], F16, tag="dt")
                order = list(range(nj - 1, -1, -1))  # high j (DVE) first
                done = set()
                halves = [
                    (nj // 2, nj, slice(k // 2, k)),
                    (0, nj // 2, slice(0, k // 2)),
                ] if nj > 1 else [(0, 1, slice(0, k))]
                emitted = set()
                for j_idx in order:
                    j = j_idx * 512
                    nc.tensor.matmul(
                        s_ps[:, j : j + 512],
                        eye_t[:],
                        sq[:, k + j : k + j + 512],
                        start=False,
                        stop=True,
                    )
                    done.add(j_idx)
                    for hi, (lo_j, hi_j, cs) in enumerate(halves):
                        if hi in emitted:
                            continue
                        if all(jj in done for jj in range(lo_j, hi_j)):
                            emitted.add(hi)
                            nc.scalar.activation(
                                out=dt[:, cs], in_=s_ps[:, cs], func=AF.Sqrt
                            )
                            nc.sync.dma_start(
                                out=d[:, doff + cs.start : doff + cs.stop],
                                in_=dt[:, cs],
                            )

            stage_a(0)
            nc.sync.dma_start(out=eye_t[:], in_=eye[:])
            for t in range(1, NT):
                stage_a(t)
                act_square(t - 1)
                stage_b(t - 1)
            act_square(NT - 1)
            stage_b(NT - 1)

    nc.compile()
    return nc


_cached_nc = None


def _get_nc():
    global _cached_nc
    if _cached_nc is None:
        _cached_nc = build_kernel()
    return _cached_nc


_AXON_SO = "/opt/axon/libaxon_pjrt.so"


def _ensure_ntff_hook():
    """Install an antenv.axon_hooks shim backed by libaxon_pjrt's NRT
    profiling C ABI, so run_bass_kernel_spmd(trace=True) works under axon."""
    try:
        from antenv.axon_hooks import get_axon_ntff_profile_hook  # noqa: F401

        return
    except ImportError:
        pass
    import contextlib
    import ctypes
    import types

    import antenv

    holder = {}
    mod = types.ModuleType("antenv.axon_hooks")
    mod.set_axon_ntff_profile_hook = lambda h: holder.__setitem__("h", h)
    mod.get_axon_ntff_profile_hook = lambda: holder.get("h")
    sys.modules["antenv.axon_hooks"] = mod
    antenv.axon_hooks = mod

    try:
        lib = ctypes.CDLL(_AXON_SO)
    except OSError:
        return
    if not hasattr(lib, "axon_start_nrt_profile"):
        return
    lib.axon_start_nrt_profile.argtypes = [
        ctypes.POINTER(ctypes.c_int64),
        ctypes.c_size_t,
    ]
    lib.axon_start_nrt_profile.restype = ctypes.c_int64
    lib.axon_stop_nrt_profile.argtypes = [ctypes.c_char_p]
    lib.axon_stop_nrt_profile.restype = ctypes.c_int64

    @contextlib.contextmanager
    def _hook(output_dir, device_ids):
        import jax

        jax.devices()
        if device_ids:
            ids = (ctypes.c_int64 * len(device_ids))(*device_ids)
            rc = lib.axon_start_nrt_profile(ids, len(device_ids))
        else:
            rc = lib.axon_start_nrt_profile(None, 0)
        if rc != 0:
            raise RuntimeError(f"axon_start_nrt_profile rc={rc}")
        try:
            yield
        finally:
            n = lib.axon_stop_nrt_profile(str(output_dir).encode())
            print(f"ntff profile: {n} file(s) written to {output_dir}")

    holder["h"] = _hook


def _host_shards(pts):
    """[N,3] f32 -> per-core [P, 3*KPL] fp16, planar per variable tile."""
    h = pts.astype(np.float16)
    # [NC, P, KPL, 3]
    g = h.reshape(NCORES, P, KPL, 3)
    out = np.empty((NCORES, P, 3 * KPL), dtype=np.float16)
    off = 0
    for k in TILES:
        blk = g[:, :, off : off + k, :].transpose(0, 1, 3, 2)  # [NC,P,3,k]
        out[:, :, 3 * off : 3 * (off + k)] = blk.reshape(NCORES, P, 3 * k)
        off += k
    return out


def run(inputs_array, trace=False, **kwargs):
    """inputs_array: [N, 3] float32. Returns (out [N] float32, BassKernelResults)."""
    pts = np.ascontiguousarray(inputs_array, dtype=np.float32)
    assert pts.shape == (N, 3), pts.shape
    shards = _host_shards(pts)
    if trace:
        _ensure_ntff_hook()
    nc = _get_nc()
    eye_np = np.eye(P, dtype=np.float16)
    in_maps = [{"x": shards[i], "eye": eye_np} for i in range(NCORES)]
    res = bass_utils.run_bass_kernel_spmd(
        nc, in_maps, core_ids=list(range(NCORES)), trace=trace, **kwargs
    )
    out = np.concatenate(
        [res.results[i]["d"].reshape(-1) for i in range(NCORES)]
    ).astype(np.float32)
    return out, res


def kernel(**inputs):
    out, _ = run(inputs["inputs"])
    return out


if __name__ == "__main__":
    rng = np.random.default_rng(0)
    pts = rng.standard_normal((N, 3)).astype(np.float32)
    out, _ = run(pts)
    q = np.abs(pts) - SIZE
    inside = np.all(q < 0, axis=1)
    d_out = np.sqrt(np.sum(np.square(np.maximum(q, 0.0)), axis=1))
    d_in = -np.max(q, axis=1)
    exp = np.where(inside, d_in, d_out)
    err = np.abs(out - exp) / np.maximum(np.abs(exp), 1e-6)
    print("max rel err:", err.max(), "mean:", err.mean())


# revision 12
# speedup vs baseline: 1.0105x; 1.0105x over previous
"""Box-SDF (CAPUDF box boundary distance) Trainium2 Bass kernel — fp16 v4.

For each 3-D point x (S = 0.4):
    a_i = |x_i|                         (ACT Abs)
    b_i = relu(a_i - S) = max(a_i,S)-S  (DVE fused 2-op tensor_scalar, 4x)
    mx  = max_i a_i                     (DVE max tree)
    u   = min(mx, S) - S                (<= 0; == -(inside distance))
    b0' = b_0 + u                       (disjoint support: exact)
    d   = sqrt(b0'^2 + b1^2 + b2^2)     (= sqrt(sum b_i^2 + u^2))

Placement (measured fp16 rates): ACT = abs + 3/4 of the b1 square +
sqrt; DVE = everything else (self-read tensor_tensor squares have no
fp16 penalty); PE sums the 3 planes via identity-matmul 512-col PSUM
chunks, consuming DVE-fed planes first so the ACT square can lag one
tile; sqrt+store run per PSUM half to shorten the drain.

Tile schedule [1024, 2048, 2048, 2048, 1024] points/partition-row:
small edge tiles cut pipeline fill/drain; flat DRAM layout.

Device I/O fp16 (host converts; L2 rel err ~4e-4 vs 2e-2 gate).
Sharding: data-parallel over points across 8 NeuronCores.
"""

import sys

import numpy as np

sys.path.insert(0, "/opt/trn_rl_repo")

import concourse.bacc as bacc  # noqa: E402
import concourse.mybir as mybir  # noqa: E402
from concourse import bass_utils  # noqa: E402
from concourse.tile import TileContext  # noqa: E402

N = 8388608
NCORES = 8
NPC = N // NCORES  # 1,048,576 points per core
P = 128
KPL = NPC // P  # 8192 points per partition lane
K = 2048  # points per partition row per tile
F3 = 3 * K
NT = KPL // K  # 4 tiles per core

SIZE = 0.4
F16 = mybir.dt.float16
F32 = mybir.dt.float32
AF = mybir.ActivationFunctionType
OP = mybir.AluOpType


def build_kernel():
    nc = bacc.Bacc(
        "TRN2",
        target_bir_lowering=False,
        debug=False,
        num_devices=NCORES,
    )
    x = nc.dram_tensor("x", [NT, P, F3], F16, kind="ExternalInput").ap()
    eye = nc.dram_tensor("eye", [P, P], F16, kind="ExternalInput").ap()
    d = nc.dram_tensor("d", [NT, P, K], F16, kind="ExternalOutput").ap()

    with TileContext(nc) as tc:
        with (
            tc.tile_pool(name="const", bufs=1) as cpool,
            tc.tile_pool(name="xtp", bufs=3) as xtp,
            tc.tile_pool(name="big", bufs=2) as big,
            tc.tile_pool(name="small", bufs=3) as small,
            tc.tile_pool(name="psum", bufs=2, space="PSUM") as pspool,
        ):
            eye_t = cpool.tile([P, P], F16)
            state = {}

            def stage_a(t):
                xt = xtp.tile([P, F3], F16, tag="xt")
                aa = big.tile([P, F3], F16, tag="aa")
                if t == 0:
                    # chunk tile 0 per-plane so ACT starts sooner
                    for c in range(3):
                        cs = slice(c * K, (c + 1) * K)
                        nc.sync.dma_start(out=xt[:, cs], in_=x[t][:, cs])
                        nc.scalar.activation(
                            out=aa[:, cs], in_=xt[:, cs], func=AF.Abs
                        )
                else:
                    nc.sync.dma_start(out=xt[:], in_=x[t])
                    nc.scalar.activation(out=aa[:], in_=xt[:], func=AF.Abs)

                # b planes: relu(a - S) (fused max+add, 4x)
                bb = big.tile([P, F3], F16, tag="bb")
                nc.vector.tensor_scalar(
                    out=bb[:],
                    in0=aa[:],
                    scalar1=SIZE,
                    scalar2=-SIZE,
                    op0=OP.max,
                    op1=OP.add,
                )
                # mx tree
                m1 = small.tile([P, K], F16, tag="m1")
                nc.vector.tensor_tensor(
                    out=m1[:], in0=aa[:, 0:K], in1=aa[:, K : 2 * K], op=OP.max
                )
                mx = small.tile([P, K], F16, tag="mx")
                nc.vector.tensor_tensor(
                    out=mx[:], in0=m1[:], in1=aa[:, 2 * K : 3 * K], op=OP.max
                )
                # u = min(mx,S)-S ; b0' = b0 + u
                u = small.tile([P, K], F16, tag="u")
                nc.vector.tensor_scalar(
                    out=u[:],
                    in0=mx[:],
                    scalar1=SIZE,
                    scalar2=-SIZE,
                    op0=OP.min,
                    op1=OP.add,
                )
                b0p = small.tile([P, K], F16, tag="b0p")
                nc.vector.tensor_tensor(
                    out=b0p[:], in0=bb[:, 0:K], in1=u[:], op=OP.add
                )

                # DVE self-squares: b0', b2, second half of b1
                sq = big.tile([P, F3], F16, tag="sq")
                nc.vector.tensor_tensor(
                    out=sq[:, 0:K], in0=b0p[:], in1=b0p[:], op=OP.mult
                )
                nc.vector.tensor_tensor(
                    out=sq[:, 2 * K : 3 * K],
                    in0=bb[:, 2 * K : 3 * K],
                    in1=bb[:, 2 * K : 3 * K],
                    op=OP.mult,
                )
                H = K // 2
                nc.vector.tensor_tensor(
                    out=sq[:, K + H : 2 * K],
                    in0=bb[:, K + H : 2 * K],
                    in1=bb[:, K + H : 2 * K],
                    op=OP.mult,
                )
                state[t] = (bb, sq)

            def stage_b(t):
                bb, sq = state.pop(t)
                H = K // 2
                nc.scalar.activation(
                    out=sq[:, K : K + H], in_=bb[:, K : K + H], func=AF.Square
                )
                s_ps = pspool.tile([P, K], F32, tag="s_ps")
                for j in range(0, K, 512):
                    for c in range(3):
                        nc.tensor.matmul(
                            s_ps[:, j : j + 512],
                            eye_t[:],
                            sq[:, c * K + j : c * K + j + 512],
                            start=(c == 0),
                            stop=(c == 2),
                        )
                dt = small.tile([P, K], F16, tag="dt")
                nc.scalar.activation(out=dt[:], in_=s_ps[:], func=AF.Sqrt)
                nc.sync.dma_start(out=d[t], in_=dt[:])

            # eye first, then PE warmup matmuls to ramp the PE clock while
            # the input DMA + first Abs fill the pipeline head
            nc.sync.dma_start(out=eye_t[:], in_=eye[:])
            wps = pspool.tile([P, K], F32, tag="s_ps")
            for w in range(10):
                nc.tensor.matmul(
                    wps[:, 0:P], eye_t[:], eye_t[:], start=True, stop=True
                )
            stage_a(0)
            for t in range(1, NT):
                stage_a(t)
                stage_b(t - 1)
            stage_b(NT - 1)

    nc.compile()
    return nc


_cached_nc = None


def _get_nc():
    global _cached_nc
    if _cached_nc is None:
        _cached_nc = build_kernel()
    return _cached_nc


_AXON_SO = "/opt/axon/libaxon_pjrt.so"


def _ensure_ntff_hook():
    """Install an antenv.axon_hooks shim backed by libaxon_pjrt's NRT
    profiling C ABI, so run_bass_kernel_spmd(trace=True) works under axon."""
    try:
        from antenv.axon_hooks import get_axon_ntff_profile_hook  # noqa: F401

        return
    except ImportError:
        pass
    import contextlib
    import ctypes
    import types

    import antenv

    holder = {}
    mod = types.ModuleType("antenv.axon_hooks")
    mod.set_axon_ntff_profile_hook = lambda h: holder.__setitem__("h", h)
    mod.get_axon_ntff_profile_hook = lambda: holder.get("h")
    sys.modules["antenv.axon_hooks"] = mod
    antenv.axon_hooks = mod

    try:
        lib = ctypes.CDLL(_AXON_SO)
    except OSError:
        return
    if not hasattr(lib, "axon_start_nrt_profile"):
        return
    lib.axon_start_nrt_profile.argtypes = [
        ctypes.POINTER(ctypes.c_int64),
        ctypes.c_size_t,
    ]
    lib.axon_start_nrt_profile.restype = ctypes.c_int64
    lib.axon_stop_nrt_profile.argtypes = [ctypes.c_char_p]
    lib.axon_stop_nrt_profile.restype = ctypes.c_int64

    @contextlib.contextmanager
    def _hook(output_dir, device_ids):
        import jax

        jax.devices()
        if device_ids:
            ids = (ctypes.c_int64 * len(device_ids))(*device_ids)
            rc = lib.axon_start_nrt_profile(ids, len(device_ids))
        else:
            rc = lib.axon_start_nrt_profile(None, 0)
        if rc != 0:
            raise RuntimeError(f"axon_start_nrt_profile rc={rc}")
        try:
            yield
        finally:
            n = lib.axon_stop_nrt_profile(str(output_dir).encode())
            print(f"ntff profile: {n} file(s) written to {output_dir}")

    holder["h"] = _hook


def _host_shards(pts):
    """[N,3] f32 -> per-core [NT, P, 3K] fp16 planar tiles."""
    h = pts.astype(np.float16)
    return np.ascontiguousarray(
        h.reshape(NCORES, NT, P, K, 3).transpose(0, 1, 2, 4, 3)
    ).reshape(NCORES, NT, P, F3)


def run(inputs_array, trace=False, **kwargs):
    """inputs_array: [N, 3] float32. Returns (out [N] float32, BassKernelResults)."""
    pts = np.ascontiguousarray(inputs_array, dtype=np.float32)
    assert pts.shape == (N, 3), pts.shape
    shards = _host_shards(pts)
    if trace:
        _ensure_ntff_hook()
    nc = _get_nc()
    eye_np = np.eye(P, dtype=np.float16)
    in_maps = [{"x": shards[i], "eye": eye_np} for i in range(NCORES)]
    res = bass_utils.run_bass_kernel_spmd(
        nc, in_maps, core_ids=list(range(NCORES)), trace=trace, **kwargs
    )
    out = np.concatenate(
        [res.results[i]["d"].reshape(-1) for i in range(NCORES)]
    ).astype(np.float32)
    return out, res


def kernel(**inputs):
    out, _ = run(inputs["inputs"])
    return out


if __name__ == "__main__":
    rng = np.random.default_rng(0)
    pts = rng.standard_normal((N, 3)).astype(np.float32)
    out, _ = run(pts)
    q = np.abs(pts) - SIZE
    inside = np.all(q < 0, axis=1)
    d_out = np.sqrt(np.sum(np.square(np.maximum(q, 0.0)), axis=1))
    d_in = -np.max(q, axis=1)
    exp = np.where(inside, d_in, d_out)
    err = np.abs(out - exp) / np.maximum(np.abs(exp), 1e-6)
    print("max rel err:", err.max(), "mean:", err.mean())


# revision 13
# speedup vs baseline: 1.0360x; 1.0252x over previous
"""Box-SDF (CAPUDF box boundary distance) Trainium2 Bass kernel — fp16 v4.

For each 3-D point x (S = 0.4):
    a_i = |x_i|                         (ACT Abs)
    b_i = relu(a_i - S) = max(a_i,S)-S  (DVE fused 2-op tensor_scalar, 4x)
    mx  = max_i a_i                     (DVE max tree)
    u   = min(mx, S) - S                (<= 0; == -(inside distance))
    b0' = b_0 + u                       (disjoint support: exact)
    d   = sqrt(b0'^2 + b1^2 + b2^2)     (= sqrt(sum b_i^2 + u^2))

Placement (measured fp16 rates): ACT = abs + 3/4 of the b1 square +
sqrt; DVE = everything else (self-read tensor_tensor squares have no
fp16 penalty); PE sums the 3 planes via identity-matmul 512-col PSUM
chunks, consuming DVE-fed planes first so the ACT square can lag one
tile; sqrt+store run per PSUM half to shorten the drain.

Tile schedule [1024, 2048, 2048, 2048, 1024] points/partition-row:
small edge tiles cut pipeline fill/drain; flat DRAM layout.

Device I/O fp16 (host converts; L2 rel err ~4e-4 vs 2e-2 gate).
Sharding: data-parallel over points across 8 NeuronCores.
"""

import sys

import numpy as np

sys.path.insert(0, "/opt/trn_rl_repo")

import concourse.bacc as bacc  # noqa: E402
import concourse.mybir as mybir  # noqa: E402
from concourse import bass_utils  # noqa: E402
from concourse.tile import TileContext  # noqa: E402

N = 8388608
NCORES = 8
NPC = N // NCORES  # 1,048,576 points per core
P = 128
KPL = NPC // P  # 8192 points per partition lane
K = 2048  # points per partition row per tile
F3 = 3 * K
NT = KPL // K  # 4 tiles per core

SIZE = 0.4
F16 = mybir.dt.float16
F32 = mybir.dt.float32
AF = mybir.ActivationFunctionType
OP = mybir.AluOpType


def build_kernel():
    nc = bacc.Bacc(
        "TRN2",
        target_bir_lowering=False,
        debug=False,
        num_devices=NCORES,
    )
    x = nc.dram_tensor("x", [NT, P, F3], F16, kind="ExternalInput").ap()
    eye = nc.dram_tensor("eye", [P, P], F16, kind="ExternalInput").ap()
    d = nc.dram_tensor("d", [NT, P, K], F16, kind="ExternalOutput").ap()

    with TileContext(nc) as tc:
        with (
            tc.tile_pool(name="const", bufs=1) as cpool,
            tc.tile_pool(name="xtp", bufs=3) as xtp,
            tc.tile_pool(name="big", bufs=2) as big,
            tc.tile_pool(name="small", bufs=3) as small,
            tc.tile_pool(name="psum", bufs=2, space="PSUM") as pspool,
        ):
            eye_t = cpool.tile([P, P], F16)
            state = {}

            def stage_a(t):
                xt = xtp.tile([P, F3], F16, tag="xt")
                aa = big.tile([P, F3], F16, tag="aa")
                if t == 0:
                    # chunk tile 0 per-plane so ACT starts sooner
                    for c in range(3):
                        cs = slice(c * K, (c + 1) * K)
                        nc.sync.dma_start(out=xt[:, cs], in_=x[t][:, cs])
                        nc.scalar.activation(
                            out=aa[:, cs], in_=xt[:, cs], func=AF.Abs
                        )
                else:
                    nc.sync.dma_start(out=xt[:], in_=x[t])
                    nc.scalar.activation(out=aa[:], in_=xt[:], func=AF.Abs)

                # b planes: relu(a - S) (fused max+add, 4x)
                bb = big.tile([P, F3], F16, tag="bb")
                nc.vector.tensor_scalar(
                    out=bb[:],
                    in0=aa[:],
                    scalar1=SIZE,
                    scalar2=-SIZE,
                    op0=OP.max,
                    op1=OP.add,
                )
                # mx tree
                m1 = small.tile([P, K], F16, tag="m1")
                nc.vector.tensor_tensor(
                    out=m1[:], in0=aa[:, 0:K], in1=aa[:, K : 2 * K], op=OP.max
                )
                mx = small.tile([P, K], F16, tag="mx")
                nc.vector.tensor_tensor(
                    out=mx[:], in0=m1[:], in1=aa[:, 2 * K : 3 * K], op=OP.max
                )
                # u = min(mx,S)-S ; b0' = b0 + u
                u = small.tile([P, K], F16, tag="u")
                nc.vector.tensor_scalar(
                    out=u[:],
                    in0=mx[:],
                    scalar1=SIZE,
                    scalar2=-SIZE,
                    op0=OP.min,
                    op1=OP.add,
                )
                b0p = small.tile([P, K], F16, tag="b0p")
                nc.vector.tensor_tensor(
                    out=b0p[:], in0=bb[:, 0:K], in1=u[:], op=OP.add
                )

                # DVE self-squares: b0', b2, second half of b1
                sq = big.tile([P, F3], F16, tag="sq")
                nc.vector.tensor_tensor(
                    out=sq[:, 0:K], in0=b0p[:], in1=b0p[:], op=OP.mult
                )
                nc.vector.tensor_tensor(
                    out=sq[:, 2 * K : 3 * K],
                    in0=bb[:, 2 * K : 3 * K],
                    in1=bb[:, 2 * K : 3 * K],
                    op=OP.mult,
                )
                state[t] = (bb, sq)

            def stage_b(t):
                bb, sq = state.pop(t)
                # ACT square: plane b1 (emitted here so abs(t+1) can slip ahead)
                nc.scalar.activation(
                    out=sq[:, K : 2 * K], in_=bb[:, K : 2 * K], func=AF.Square
                )
                s_ps = pspool.tile([P, K], F32, tag="s_ps")
                for j in range(0, K, 512):
                    for c in range(3):
                        nc.tensor.matmul(
                            s_ps[:, j : j + 512],
                            eye_t[:],
                            sq[:, c * K + j : c * K + j + 512],
                            start=(c == 0),
                            stop=(c == 2),
                        )
                dt = small.tile([P, K], F16, tag="dt")
                nc.scalar.activation(out=dt[:], in_=s_ps[:], func=AF.Sqrt)
                nc.sync.dma_start(out=d[t], in_=dt[:])

            stage_a(0)
            nc.sync.dma_start(out=eye_t[:], in_=eye[:])
            for t in range(1, NT):
                stage_a(t)
                stage_b(t - 1)
            stage_b(NT - 1)

    nc.compile()
    return nc


_cached_nc = None


def _get_nc():
    global _cached_nc
    if _cached_nc is None:
        _cached_nc = build_kernel()
    return _cached_nc


_AXON_SO = "/opt/axon/libaxon_pjrt.so"


def _ensure_ntff_hook():
    """Install an antenv.axon_hooks shim backed by libaxon_pjrt's NRT
    profiling C ABI, so run_bass_kernel_spmd(trace=True) works under axon."""
    try:
        from antenv.axon_hooks import get_axon_ntff_profile_hook  # noqa: F401

        return
    except ImportError:
        pass
    import contextlib
    import ctypes
    import types

    import antenv

    holder = {}
    mod = types.ModuleType("antenv.axon_hooks")
    mod.set_axon_ntff_profile_hook = lambda h: holder.__setitem__("h", h)
    mod.get_axon_ntff_profile_hook = lambda: holder.get("h")
    sys.modules["antenv.axon_hooks"] = mod
    antenv.axon_hooks = mod

    try:
        lib = ctypes.CDLL(_AXON_SO)
    except OSError:
        return
    if not hasattr(lib, "axon_start_nrt_profile"):
        return
    lib.axon_start_nrt_profile.argtypes = [
        ctypes.POINTER(ctypes.c_int64),
        ctypes.c_size_t,
    ]
    lib.axon_start_nrt_profile.restype = ctypes.c_int64
    lib.axon_stop_nrt_profile.argtypes = [ctypes.c_char_p]
    lib.axon_stop_nrt_profile.restype = ctypes.c_int64

    @contextlib.contextmanager
    def _hook(output_dir, device_ids):
        import jax

        jax.devices()
        if device_ids:
            ids = (ctypes.c_int64 * len(device_ids))(*device_ids)
            rc = lib.axon_start_nrt_profile(ids, len(device_ids))
        else:
            rc = lib.axon_start_nrt_profile(None, 0)
        if rc != 0:
            raise RuntimeError(f"axon_start_nrt_profile rc={rc}")
        try:
            yield
        finally:
            n = lib.axon_stop_nrt_profile(str(output_dir).encode())
            print(f"ntff profile: {n} file(s) written to {output_dir}")

    holder["h"] = _hook


def _host_shards(pts):
    """[N,3] f32 -> per-core [NT, P, 3K] fp16 planar tiles."""
    h = pts.astype(np.float16)
    return np.ascontiguousarray(
        h.reshape(NCORES, NT, P, K, 3).transpose(0, 1, 2, 4, 3)
    ).reshape(NCORES, NT, P, F3)


def run(inputs_array, trace=False, **kwargs):
    """inputs_array: [N, 3] float32. Returns (out [N] float32, BassKernelResults)."""
    pts = np.ascontiguousarray(inputs_array, dtype=np.float32)
    assert pts.shape == (N, 3), pts.shape
    shards = _host_shards(pts)
    if trace:
        _ensure_ntff_hook()
    nc = _get_nc()
    eye_np = np.eye(P, dtype=np.float16)
    in_maps = [{"x": shards[i], "eye": eye_np} for i in range(NCORES)]
    res = bass_utils.run_bass_kernel_spmd(
        nc, in_maps, core_ids=list(range(NCORES)), trace=trace, **kwargs
    )
    out = np.concatenate(
        [res.results[i]["d"].reshape(-1) for i in range(NCORES)]
    ).astype(np.float32)
    return out, res


def kernel(**inputs):
    out, _ = run(inputs["inputs"])
    return out


if __name__ == "__main__":
    rng = np.random.default_rng(0)
    pts = rng.standard_normal((N, 3)).astype(np.float32)
    out, _ = run(pts)
    q = np.abs(pts) - SIZE
    inside = np.all(q < 0, axis=1)
    d_out = np.sqrt(np.sum(np.square(np.maximum(q, 0.0)), axis=1))
    d_in = -np.max(q, axis=1)
    exp = np.where(inside, d_in, d_out)
    err = np.abs(out - exp) / np.maximum(np.abs(exp), 1e-6)
    print("max rel err:", err.max(), "mean:", err.mean())


# revision 14
# speedup vs baseline: 1.0897x; 1.0518x over previous
"""Box-SDF (CAPUDF box boundary distance) Trainium2 Bass kernel — fp16 v4.

For each 3-D point x (S = 0.4):
    a_i = |x_i|                         (ACT Abs)
    b_i = relu(a_i - S) = max(a_i,S)-S  (DVE fused 2-op tensor_scalar, 4x)
    mx  = max_i a_i                     (DVE max tree)
    u   = min(mx, S) - S                (<= 0; == -(inside distance))
    b0' = b_0 + u                       (disjoint support: exact)
    d   = sqrt(b0'^2 + b1^2 + b2^2)     (= sqrt(sum b_i^2 + u^2))

Placement (measured fp16 rates): ACT = abs + 3/4 of the b1 square +
sqrt; DVE = everything else (self-read tensor_tensor squares have no
fp16 penalty); PE sums the 3 planes via identity-matmul 512-col PSUM
chunks, consuming DVE-fed planes first so the ACT square can lag one
tile; sqrt+store run per PSUM half to shorten the drain.

Tile schedule [1024, 2048, 2048, 2048, 1024] points/partition-row:
small edge tiles cut pipeline fill/drain; flat DRAM layout.

Device I/O fp16 (host converts; L2 rel err ~4e-4 vs 2e-2 gate).
Sharding: data-parallel over points across 8 NeuronCores.
"""

import sys

import numpy as np

sys.path.insert(0, "/opt/trn_rl_repo")

import concourse.bacc as bacc  # noqa: E402
import concourse.mybir as mybir  # noqa: E402
from concourse import bass_utils  # noqa: E402
from concourse.tile import TileContext  # noqa: E402

N = 8388608
NCORES = 8
NPC = N // NCORES  # 1,048,576 points per core
P = 128
KPL = NPC // P  # 8192 points per partition lane
K = 2048  # points per partition row per tile
F3 = 3 * K
NT = KPL // K  # 4 tiles per core

SIZE = 0.4
F16 = mybir.dt.float16
F32 = mybir.dt.float32
AF = mybir.ActivationFunctionType
OP = mybir.AluOpType


def build_kernel():
    nc = bacc.Bacc(
        "TRN2",
        target_bir_lowering=False,
        debug=False,
        num_devices=NCORES,
    )
    x = nc.dram_tensor("x", [NT, P, F3], F16, kind="ExternalInput").ap()
    eye = nc.dram_tensor("eye", [P, P], F16, kind="ExternalInput").ap()
    d = nc.dram_tensor("d", [NT, P, K], F16, kind="ExternalOutput").ap()

    with TileContext(nc) as tc:
        with (
            tc.tile_pool(name="const", bufs=1) as cpool,
            tc.tile_pool(name="xtp", bufs=3) as xtp,
            tc.tile_pool(name="big", bufs=2) as big,
            tc.tile_pool(name="bigb", bufs=3) as bigb,
            tc.tile_pool(name="small", bufs=3) as small,
            tc.tile_pool(name="psum", bufs=2, space="PSUM") as pspool,
        ):
            eye_t = cpool.tile([P, P], F16)
            state = {}

            def stage_a(t):
                xt = xtp.tile([P, F3], F16, tag="xt")
                aa = big.tile([P, F3], F16, tag="aa")
                if t == 0:
                    # chunk tile 0 per-plane so ACT starts sooner
                    for c in range(3):
                        cs = slice(c * K, (c + 1) * K)
                        nc.sync.dma_start(out=xt[:, cs], in_=x[t][:, cs])
                        nc.scalar.activation(
                            out=aa[:, cs], in_=xt[:, cs], func=AF.Abs
                        )
                else:
                    nc.sync.dma_start(out=xt[:], in_=x[t])
                    nc.scalar.activation(out=aa[:], in_=xt[:], func=AF.Abs)

                # b planes: relu(a - S) (fused max+add, 4x)
                bb = bigb.tile([P, F3], F16, tag="bb")
                nc.vector.tensor_scalar(
                    out=bb[:],
                    in0=aa[:],
                    scalar1=SIZE,
                    scalar2=-SIZE,
                    op0=OP.max,
                    op1=OP.add,
                )
                # mx tree
                m1 = small.tile([P, K], F16, tag="m1")
                nc.vector.tensor_tensor(
                    out=m1[:], in0=aa[:, 0:K], in1=aa[:, K : 2 * K], op=OP.max
                )
                mx = small.tile([P, K], F16, tag="mx")
                nc.vector.tensor_tensor(
                    out=mx[:], in0=m1[:], in1=aa[:, 2 * K : 3 * K], op=OP.max
                )
                # u = min(mx,S)-S ; b0' = b0 + u
                u = small.tile([P, K], F16, tag="u")
                nc.vector.tensor_scalar(
                    out=u[:],
                    in0=mx[:],
                    scalar1=SIZE,
                    scalar2=-SIZE,
                    op0=OP.min,
                    op1=OP.add,
                )
                b0p = small.tile([P, K], F16, tag="b0p")
                nc.vector.tensor_tensor(
                    out=b0p[:], in0=bb[:, 0:K], in1=u[:], op=OP.add
                )

                # DVE self-squares: b0', b2, second half of b1
                sq = bigb.tile([P, F3], F16, tag="sq")
                nc.vector.tensor_tensor(
                    out=sq[:, 0:K], in0=b0p[:], in1=b0p[:], op=OP.mult
                )
                nc.vector.tensor_tensor(
                    out=sq[:, 2 * K : 3 * K],
                    in0=bb[:, 2 * K : 3 * K],
                    in1=bb[:, 2 * K : 3 * K],
                    op=OP.mult,
                )
                state[t] = (bb, sq)

            def stage_b(t):
                bb, sq = state.pop(t)
                # ACT square: plane b1 (emitted here so abs(t+1) can slip ahead)
                nc.scalar.activation(
                    out=sq[:, K : 2 * K], in_=bb[:, K : 2 * K], func=AF.Square
                )
                s_ps = pspool.tile([P, K], F32, tag="s_ps")
                for j in range(0, K, 512):
                    for c in range(3):
                        nc.tensor.matmul(
                            s_ps[:, j : j + 512],
                            eye_t[:],
                            sq[:, c * K + j : c * K + j + 512],
                            start=(c == 0),
                            stop=(c == 2),
                        )
                dt = small.tile([P, K], F16, tag="dt")
                nc.scalar.activation(out=dt[:], in_=s_ps[:], func=AF.Sqrt)
                nc.sync.dma_start(out=d[t], in_=dt[:])

            stage_a(0)
            nc.sync.dma_start(out=eye_t[:], in_=eye[:])
            for t in range(1, NT):
                stage_a(t)
                stage_b(t - 1)
            stage_b(NT - 1)

    nc.compile()
    return nc


_cached_nc = None


def _get_nc():
    global _cached_nc
    if _cached_nc is None:
        _cached_nc = build_kernel()
    return _cached_nc


_AXON_SO = "/opt/axon/libaxon_pjrt.so"


def _ensure_ntff_hook():
    """Install an antenv.axon_hooks shim backed by libaxon_pjrt's NRT
    profiling C ABI, so run_bass_kernel_spmd(trace=True) works under axon."""
    try:
        from antenv.axon_hooks import get_axon_ntff_profile_hook  # noqa: F401

        return
    except ImportError:
        pass
    import contextlib
    import ctypes
    import types

    import antenv

    holder = {}
    mod = types.ModuleType("antenv.axon_hooks")
    mod.set_axon_ntff_profile_hook = lambda h: holder.__setitem__("h", h)
    mod.get_axon_ntff_profile_hook = lambda: holder.get("h")
    sys.modules["antenv.axon_hooks"] = mod
    antenv.axon_hooks = mod

    try:
        lib = ctypes.CDLL(_AXON_SO)
    except OSError:
        return
    if not hasattr(lib, "axon_start_nrt_profile"):
        return
    lib.axon_start_nrt_profile.argtypes = [
        ctypes.POINTER(ctypes.c_int64),
        ctypes.c_size_t,
    ]
    lib.axon_start_nrt_profile.restype = ctypes.c_int64
    lib.axon_stop_nrt_profile.argtypes = [ctypes.c_char_p]
    lib.axon_stop_nrt_profile.restype = ctypes.c_int64

    @contextlib.contextmanager
    def _hook(output_dir, device_ids):
        import jax

        jax.devices()
        if device_ids:
            ids = (ctypes.c_int64 * len(device_ids))(*device_ids)
            rc = lib.axon_start_nrt_profile(ids, len(device_ids))
        else:
            rc = lib.axon_start_nrt_profile(None, 0)
        if rc != 0:
            raise RuntimeError(f"axon_start_nrt_profile rc={rc}")
        try:
            yield
        finally:
            n = lib.axon_stop_nrt_profile(str(output_dir).encode())
            print(f"ntff profile: {n} file(s) written to {output_dir}")

    holder["h"] = _hook


def _host_shards(pts):
    """[N,3] f32 -> per-core [NT, P, 3K] fp16 planar tiles."""
    h = pts.astype(np.float16)
    return np.ascontiguousarray(
        h.reshape(NCORES, NT, P, K, 3).transpose(0, 1, 2, 4, 3)
    ).reshape(NCORES, NT, P, F3)


def run(inputs_array, trace=False, **kwargs):
    """inputs_array: [N, 3] float32. Returns (out [N] float32, BassKernelResults)."""
    pts = np.ascontiguousarray(inputs_array, dtype=np.float32)
    assert pts.shape == (N, 3), pts.shape
    shards = _host_shards(pts)
    if trace:
        _ensure_ntff_hook()
    nc = _get_nc()
    eye_np = np.eye(P, dtype=np.float16)
    in_maps = [{"x": shards[i], "eye": eye_np} for i in range(NCORES)]
    res = bass_utils.run_bass_kernel_spmd(
        nc, in_maps, core_ids=list(range(NCORES)), trace=trace, **kwargs
    )
    out = np.concatenate(
        [res.results[i]["d"].reshape(-1) for i in range(NCORES)]
    ).astype(np.float32)
    return out, res


def kernel(**inputs):
    out, _ = run(inputs["inputs"])
    return out


if __name__ == "__main__":
    rng = np.random.default_rng(0)
    pts = rng.standard_normal((N, 3)).astype(np.float32)
    out, _ = run(pts)
    q = np.abs(pts) - SIZE
    inside = np.all(q < 0, axis=1)
    d_out = np.sqrt(np.sum(np.square(np.maximum(q, 0.0)), axis=1))
    d_in = -np.max(q, axis=1)
    exp = np.where(inside, d_in, d_out)
    err = np.abs(out - exp) / np.maximum(np.abs(exp), 1e-6)
    print("max rel err:", err.max(), "mean:", err.mean())
